# revision 16
# baseline (speedup 1.0000x reference)
"""Memory-augmented attention kernel for Trainium2 (Bass/Tile), 8-core data parallel.

Reference computation (per row b of B=32768, D=512, K=5):
    q' = query@Wq + bq
    k  = mem@Wk + bk ; v = mem@Wv + bv
    scores = (q'.k_j)/sqrt(D) masked-softmax -> w
    mem_out = (sum_j w_j v_j)@Wo + bo
    gate = sigmoid([query, mem_out]@Wg + bg); conf = sigmoid(max_sim - 0.7)
    out = LN(query + gate*conf*mem_out) * ln_g + ln_b

Algebraic refactoring (all biases zero / identity LN affine in this problem;
a numpy fallback covers the general case):
    scores_bk = m_bk . (query_b @ (Wq Wk^T / sqrt(D)))
    mem_b     = (sum_k w_bk m_bk) @ (Wv Wo)
    gate_b    = 1/(1+exp(-(q.g1 + rsum * mcomb.(WvWo g2))))

v2 design (vs the f32 baseline):
  - All HBM I/O in bf16 (q, m, host-pretransposed qT, output) -> ~2x less DMA.
  - Softmax without max-subtraction: scores are O(1); mask penalty -60.
  - mcomb computed TRANSPOSED directly on PE: matmul(lhsT=m_k chunk,
    rhs=diag(w_k)) accumulated over k gives mcT = (sum_k w_k m_k)^T without
    any separate transpose pass.
  - qT supplied by the host in tile-transposed layout -> no PE transposes at all.
  - -g1 / -(Wvo g2) folded as a 513th column of the weight mats -> gate dots
    ride along the big matmuls with the same stationary weights.
  - Per-engine per-tile budget (est): PE ~3.5us, DVE ~3.0us, ACT ~2.7us, GP low.

This container's walrus build only encodes one sync-wait per instruction;
see _install_tile_patches.
"""

import numpy as np

B, D, K = 32768, 512, 5
N_CORES = 8
ROWS = B // N_CORES        # rows per core
P = 128                    # partitions
NT_FULL = ROWS // P        # tiles per core (32)
NCH = D // P               # 128-contraction chunks (4)
SCALE = float(D) ** -0.5
PEN = 60.0                 # mask penalty (scores are O(1), exp(-55) == 0)
LN_EPS = 1e-5
SIM_THRESH = 0.7

_CACHE = {}

TRACE = False              # set by test harness to collect a HW profile
LAST_RESULTS = None        # BassKernelResults of the last run (for profiling)


def _install_tile_patches():
    """Work around two walrus limitations in this container:
    - instructions accept very few sync-wait slots: split the kernel-tail
      drain (which Tile loads with one wait per outstanding semaphore) into
      a chain of single-wait drains;
    - EVENT_SEMAPHORE_RANGE_CLEAR is not encodable: skip the on-device sem
      clear (each kernel() call executes a freshly loaded NEFF) while keeping
      the allocator bookkeeping.
    """
    import concourse.tile as tile
    from concourse.vector_clock import ScopedClock

    if getattr(tile.TileContext._drain_and_barrier, "_patched", False):
        return

    def patched(self, tick_clock, wait_clock):
        import bass_rust

        nc = self.nc
        drain_inst = nc.sync.drain()
        wait_clock.add_sem_waits(
            drain_inst.ins, ScopedClock({None: tick_clock.global_clock})
        )
        si = drain_inst.ins.sync_info
        waits = list(si.on_wait) if si is not None and si.on_wait else []
        if len(waits) > 1:
            drain_inst.ins.sync_info = bass_rust.SyncInfo(
                on_wait=waits[:1], on_update=list(si.on_update or [])
            )
            for w in waits[1:]:
                d2 = nc.sync.drain()
                d2.ins.sync_info = bass_rust.SyncInfo(on_wait=[w], on_update=[])
        nc.all_engine_barrier()
        assert self.sems is not None
        popped = nc._tile_sem_poison_stack.pop()
        assert popped is self._sem_poison
        sems = list(self.sems.allocated().values())
        sem_nums = [s.num for s in sems]
        nc._state.prepend_free_semaphores(sem_nums)
        for poison_set in nc._tile_sem_poison_stack:
            poison_set.update(sem_nums)
        nc.all_engine_barrier()

    patched._patched = True
    tile.TileContext._drain_and_barrier = patched

    # This walrus build accepts at most one sync-wait per instruction:
    # at commit time, peel off extra waits onto single-wait nop/drain
    # instructions inserted just before the owner.
    _orig_commit = tile.TileContext._commit_instruction

    def commit_patched(self, inst, lazy_reg_writes=True):
        import bass_rust
        from concourse import mybir

        si = inst.sync_info
        if si is not None and si.on_wait and len(si.on_wait) > 1:
            waits = list(si.on_wait)
            inst.sync_info = bass_rust.SyncInfo(
                on_wait=waits[-1:], on_update=list(si.on_update or [])
            )
            for w in waits[:-1]:
                eng = self.nc.engines[inst.engine]
                if not hasattr(eng, "engine_nop"):
                    nop = mybir.InstDrain(
                        name=self.nc.get_next_instruction_name(), ins=[], outs=[]
                    )
                    nop.engine = inst.engine
                else:
                    # sequencer-only ENGINE_NOP: carries the wait without
                    # flushing the compute pipeline the way a drain does
                    nop = eng.engine_nop().ins
                nop.sync_info = bass_rust.SyncInfo(on_wait=[w], on_update=[])
                self._add_instruction(nop)
        return _orig_commit(self, inst, lazy_reg_writes)

    tile.TileContext._commit_instruction = commit_patched


def _build(ntiles=NT_FULL):
    import concourse.bass as bass
    import concourse.tile as tile
    from concourse import mybir

    _install_tile_patches()

    f32 = mybir.dt.float32
    bf16 = mybir.dt.bfloat16
    u8 = mybir.dt.uint8
    AF = mybir.ActivationFunctionType
    OP = mybir.AluOpType

    rows = ntiles * P
    rD = 1.0 / float(D)

    nc = bass.Bass()
    mqt_d = nc.declare_dram_parameter(
        "mqt", [rows, (K + 2) * D], bf16, isOutput=False
    )
    pen_d = nc.declare_dram_parameter("pen", [P, ntiles * K], bf16, isOutput=False)
    rconf_d = nc.declare_dram_parameter("rconf", [P, ntiles], f32, isOutput=False)
    wqk_d = nc.declare_dram_parameter("wqk", [D, D + 2], bf16, isOutput=False)
    wvo_d = nc.declare_dram_parameter("wvo", [D, D + 2], bf16, isOutput=False)
    id_d = nc.declare_dram_parameter("ident", [P, P], bf16, isOutput=False)
    o_d = nc.declare_dram_parameter("o", [rows, D], bf16, isOutput=True)

    mqt_t = mqt_d.rearrange("(t p) d -> t p d", p=P)
    o_t = o_d.rearrange("(t p) d -> t p d", p=P)

    with tile.TileContext(nc) as tc:
        with (
            tc.tile_pool(name="consts", bufs=1) as consts,
            tc.tile_pool(name="mload", bufs=7) as mload,
            tc.tile_pool(name="work", bufs=5) as work,
            tc.tile_pool(name="smalls", bufs=8) as smalls,
            tc.tile_pool(name="ptmp", bufs=3, space="PSUM") as ptmp,
            tc.tile_pool(name="pmem", bufs=3, space="PSUM") as pmem,
            tc.tile_pool(name="pqg", bufs=1, space="PSUM") as pqg,
            tc.tile_pool(name="pmg", bufs=1, space="PSUM") as pmg,
        ):
            # ---- constants, loaded once ----
            wqk_sb = consts.tile([P, NCH, D + 2], bf16)
            nc.sync.dma_start(out=wqk_sb, in_=wqk_d.rearrange("(c p) e -> p c e", p=P))
            wvo_sb = consts.tile([P, NCH, D + 2], bf16)
            nc.sync.dma_start(out=wvo_sb, in_=wvo_d.rearrange("(c p) e -> p c e", p=P))
            identb = consts.tile([P, P], bf16)
            nc.sync.dma_start(out=identb, in_=id_d[:, :])

            pen_all = consts.tile([P, ntiles, K], bf16)
            nc.sync.dma_start(
                out=pen_all, in_=pen_d.rearrange("p (t k) -> p t k", k=K)
            )
            rconf_all = consts.tile([P, ntiles], f32)
            nc.sync.dma_start(out=rconf_all, in_=rconf_d[:, :])

            epsc = consts.tile([P, 1], f32)
            nc.vector.memset(epsc, LN_EPS)


            # Per-tile live state; 5-stage software pipeline (lag 4).
            st = {}

            def dma_in(t):
                s = st.setdefault(t, {})
                mqt = mload.tile([P, (K + 2) * D], bf16, tag="mqt", name="mqt")
                nc.sync.dma_start(out=mqt, in_=mqt_t[t])
                s["m"] = mqt[:, 0:K * D]
                s["q"] = mqt[:, K * D:(K + 1) * D]
                s["qT"] = mqt[:, (K + 1) * D:(K + 2) * D]

            def stage_a(t):
                # t' = q@Wqk (row-major, via host-transposed qT) ; nqdot = -q.g1
                s = st[t]
                psum_t = ptmp.tile([P, D], f32, tag="ptmp", name="psum_t")
                psum_qg = pqg.tile([P, 2], f32, tag="pqg", name="psum_qg")
                for c in range(NCH):
                    sl = slice(c * P, (c + 1) * P)
                    nc.tensor.matmul(
                        psum_t,
                        lhsT=s["qT"][:, sl],
                        rhs=wqk_sb[:, c, 0:D],
                        start=(c == 0), stop=(c == NCH - 1),
                    )
                    nc.tensor.matmul(
                        psum_qg,
                        lhsT=s["qT"][:, sl],
                        rhs=wqk_sb[:, c, D:D + 2],
                        start=(c == 0), stop=(c == NCH - 1),
                    )
                tb = work.tile([P, D], bf16, tag="t_bf", name="t_bf")
                nc.scalar.copy(out=tb, in_=psum_t)
                s["t_bf"] = tb
                nq = smalls.tile([P, 2], f32, tag="nqdot", name="nqdot")
                nc.scalar.copy(out=nq, in_=psum_qg)
                s["nqdot"] = nq

            def stage_b(t):
                # raw_k = m_k . t'  (5x STT with accumulate) ; scores = raw + pen
                s = st[t]
                raw = smalls.tile([P, K], f32, tag="raw", name="raw")
                scratch = work.tile([P, D], bf16, tag="scratch", name="scratch")
                for k in range(K):
                    nc.vector.scalar_tensor_tensor(
                        out=scratch,
                        in0=s["m"][:, k * D:(k + 1) * D],
                        scalar=1.0,
                        in1=s["t_bf"],
                        op0=OP.mult, op1=OP.mult,
                        accum_out=raw[:, k:k + 1],
                    )
                sc = smalls.tile([P, K], f32, tag="scores", name="scores")
                nc.vector.tensor_tensor(
                    out=sc, in0=raw, in1=pen_all[:, t, :], op=OP.add
                )
                s["scores"] = sc

            def stage_c(t):
                # w = exp(scores) (unnormalized); rsum = 1/sum(w); diag(w_k) tiles
                s = st[t]
                w = smalls.tile([P, K], f32, tag="w", name="wtile")
                sumexp = smalls.tile([P, 1], f32, tag="sumexp", name="sumexp")
                nc.scalar.activation(
                    out=w, in_=s["scores"], func=AF.Exp, accum_out=sumexp
                )
                rsum = smalls.tile([P, 1], f32, tag="rsum", name="rsum")
                nc.vector.reciprocal(out=rsum, in_=sumexp)
                s["rsum"] = rsum
                xg = smalls.tile([P, 1], f32, tag="xg", name="xg")
                nc.gpsimd.tensor_tensor(
                    out=xg, in0=sumexp, in1=rconf_all[:, t:t + 1], op=OP.mult
                )
                s["xg"] = xg
                dk = work.tile([P, K, P], bf16, tag="diag", name="diag")
                nc.gpsimd.tensor_tensor(
                    out=dk[:, :, :],
                    in0=identb[:, :].rearrange(
                        "p (o j) -> p o j", o=1).broadcast_to([P, K, P]),
                    in1=w[:, :].rearrange(
                        "p (k o) -> p k o", o=1).broadcast_to([P, K, P]),
                    op=OP.mult,
                )
                s["dk"] = dk

            def stage_d(t):
                # mcT = (sum_k w_k m_k)^T via matmul(lhsT=m chunk, rhs=diag(w_k));
                # mem = mcomb@Wvo ; nmdot = -mcomb.g2'
                s = st[t]
                psum_mct = ptmp.tile([P, D], f32, tag="ptmp", name="psum_mct")
                for c in range(NCH):
                    sl = slice(c * P, (c + 1) * P)
                    for k in range(K):
                        nc.tensor.matmul(
                            psum_mct[:, sl],
                            lhsT=s["m"][:, k * D + c * P: k * D + (c + 1) * P],
                            rhs=s["dk"][:, k, :],
                            start=(k == 0), stop=(k == K - 1),
                        )
                mct = work.tile([P, D], bf16, tag="mct", name="mct")
                nc.scalar.copy(out=mct, in_=psum_mct)

                pm = pmem.tile([P, D], f32, tag="pmem", name="psum_mem")
                pmgt = pmg.tile([P, 2], f32, tag="pmg", name="psum_mg")
                for c in range(NCH):
                    sl = slice(c * P, (c + 1) * P)
                    nc.tensor.matmul(
                        pm,
                        lhsT=mct[:, sl],
                        rhs=wvo_sb[:, c, 0:D],
                        start=(c == 0), stop=False,
                    )
                    nc.tensor.matmul(
                        pmgt,
                        lhsT=mct[:, sl],
                        rhs=wvo_sb[:, c, D:D + 2],
                        start=(c == 0), stop=(c == NCH - 1),
                    )
                # gate: ge = exp(-(qdot + rsum*mdot));
                # 1/s = (1+ge)*sumexp/conf = ge*X + X with X = sumexp/conf
                ge = smalls.tile([P, 1], f32, tag="ge", name="ge")
                nc.scalar.activation(
                    out=ge, in_=pmgt[:, 0:1], func=AF.Exp,
                    bias=s["nqdot"][:, 0:1], scale=s["rsum"],
                )
                rs = smalls.tile([P, 1], f32, tag="rs", name="rs")
                nc.vector.tensor_scalar(
                    out=rs, in0=ge, scalar1=s["xg"][:, 0:1],
                    scalar2=s["xg"][:, 0:1], op0=OP.mult, op1=OP.add,
                )
                ds = work.tile([P, P], bf16, tag="ds", name="ds")
                nc.vector.tensor_scalar(
                    out=ds, in0=identb, scalar1=rs[:, 0:1],
                    scalar2=None, op0=OP.mult,
                )
                s["ds"] = ds
                # rowsum(x) = qsum/s + memsum, both dot-columns of the GEMMs
                rowsum = smalls.tile([P, 1], f32, tag="rowsum", name="rowsum")
                nc.vector.tensor_scalar(
                    out=rowsum, in0=rs, scalar1=s["nqdot"][:, 1:2],
                    scalar2=pmgt[:, 1:2], op0=OP.mult, op1=OP.add,
                )
                negmu2 = smalls.tile([P, 1], f32, tag="negmu2", name="negmu2")
                nc.vector.tensor_scalar(
                    out=negmu2, in0=rowsum, scalar1=rowsum[:, 0:1],
                    scalar2=-rD * rD, op0=OP.mult, op1=OP.mult,
                )
                nrw = smalls.tile([P, 1], f32, tag="nrw", name="nrw")
                nc.vector.tensor_scalar(
                    out=nrw, in0=rowsum, scalar1=-rD, scalar2=None, op0=OP.mult
                )
                s["negmu2"] = negmu2
                s["nrw"] = nrw
                s["pmem"] = pm

            def stage_e(t):
                # x = q/s + mem (in PSUM); LN(x) == LN(q + s*mem).
                # The q/s residual closes the PSUM group opened two
                # iterations earlier, decoupling PE from the gate chain.
                s = st.pop(t)
                nc.tensor.matmul(
                    s["pmem"], lhsT=s["ds"], rhs=s["q"], start=False, stop=True,
                )
                sqscr = work.tile([P, D], bf16, tag="sqscr", name="sqscr")
                sumsq = smalls.tile([P, 1], f32, tag="sumsq", name="sumsq")
                nc.scalar.activation(
                    out=sqscr, in_=s["pmem"], func=AF.Square, accum_out=sumsq
                )
                # LN tail entirely on ACT: var -> ln -> rstd -> -mu*rstd -> apply
                varc = smalls.tile([P, 1], f32, tag="varc", name="varc")
                nc.scalar.activation(
                    out=varc, in_=sumsq, func=AF.Identity,
                    bias=s["negmu2"], scale=rD,
                )
                lnv = smalls.tile([P, 1], f32, tag="lnv", name="lnv")
                nc.scalar.activation(
                    out=lnv, in_=varc, func=AF.Ln, bias=epsc, scale=1.0
                )
                rstd = smalls.tile([P, 1], f32, tag="rstd", name="rstd")
                nc.scalar.activation(out=rstd, in_=lnv, func=AF.Exp, scale=-0.5)
                nmr = smalls.tile([P, 1], f32, tag="nmr", name="nmr")
                nc.scalar.activation(
                    out=nmr, in_=rstd, func=AF.Copy, scale=s["nrw"][:, 0:1]
                )
                outf = work.tile([P, D], bf16, tag="outf", name="outf")
                nc.scalar.activation(
                    out=outf, in_=s["pmem"], func=AF.Identity,
                    bias=nmr, scale=rstd,
                )
                nc.gpsimd.dma_start(out=o_t[t], in_=outf)

            dma_in(0)
            for i in range(ntiles + 5):
                if i + 1 < ntiles:
                    dma_in(i + 1)
                if i < ntiles:
                    stage_a(i)
                if 0 <= i - 5:
                    stage_e(i - 5)
                if 0 <= i - 3 <= ntiles - 1:
                    stage_d(i - 3)
                if 0 <= i - 2 <= ntiles - 1:
                    stage_c(i - 2)
                if 0 <= i - 1 <= ntiles - 1:
                    stage_b(i - 1)

    return nc


def _numpy_fallback(query, retrieved_memories, similarities, mask,
                    Wq, bq, Wk, bk, Wv, bv, Wo, bo, Wg, bg, ln_g, ln_b):
    x = query.astype(np.float64)
    m = retrieved_memories.astype(np.float64)
    q = x @ Wq + bq
    k = np.einsum("bkd,de->bke", m, Wk.astype(np.float64)) + bk
    v = np.einsum("bkd,de->bke", m, Wv.astype(np.float64)) + bv
    scores = np.einsum("bd,bkd->bk", q, k) * (D ** -0.5)
    scores = np.where(mask, scores, -np.inf)
    sm = scores - scores.max(-1, keepdims=True)
    w = np.exp(sm)
    w /= w.sum(-1, keepdims=True)
    w = np.where(mask, w, 0.0)
    mem = np.einsum("bk,bkd->bd", w, v) @ Wo + bo
    gate = 1 / (1 + np.exp(-(np.concatenate([x, mem], -1) @ Wg + bg)))
    conf = 1 / (1 + np.exp(-(similarities.max(-1, keepdims=True) - SIM_THRESH)))
    out = x + (gate * conf) * mem
    mu = out.mean(-1, keepdims=True)
    var = ((out - mu) ** 2).mean(-1, keepdims=True)
    out = (out - mu) / np.sqrt(var + LN_EPS) * ln_g + ln_b
    return out.astype(np.float32)


def kernel(**inputs):
    global LAST_RESULTS
    query = np.ascontiguousarray(np.asarray(inputs["query"], dtype=np.float32))
    mem = np.ascontiguousarray(
        np.asarray(inputs["retrieved_memories"], dtype=np.float32)
    )
    sims = np.ascontiguousarray(np.asarray(inputs["similarities"], dtype=np.float32))
    mask = np.asarray(inputs["mask"])
    Wq = np.asarray(inputs["Wq"], dtype=np.float64)
    Wk = np.asarray(inputs["Wk"], dtype=np.float64)
    Wv = np.asarray(inputs["Wv"], dtype=np.float64)
    Wo = np.asarray(inputs["Wo"], dtype=np.float64)
    Wg = np.asarray(inputs["Wg"], dtype=np.float64)

    # The device kernel folds all-zero biases / identity LN affine away.
    nontrivial = (
        any(np.any(np.asarray(inputs[n])) for n in ("bq", "bk", "bv", "bo", "bg"))
        or np.any(np.asarray(inputs["ln_b"]))
        or np.any(np.asarray(inputs["ln_g"]) != 1.0)
    )
    if nontrivial or query.shape != (B, D):
        return _numpy_fallback(
            query, mem, sims, mask, Wq=Wq, bq=np.asarray(inputs["bq"]),
            Wk=Wk, bk=np.asarray(inputs["bk"]), Wv=Wv, bv=np.asarray(inputs["bv"]),
            Wo=Wo, bo=np.asarray(inputs["bo"]), Wg=Wg, bg=np.asarray(inputs["bg"]),
            ln_g=np.asarray(inputs["ln_g"]), ln_b=np.asarray(inputs["ln_b"]),
        )

    import ml_dtypes
    bf = ml_dtypes.bfloat16
    wqk64 = (Wq @ Wk.T) * (float(D) ** -0.5)
    wvo64 = Wv @ Wo
    wqk_ext = np.empty((D, D + 2), dtype=bf)
    wqk_ext[:, :D] = wqk64.astype(bf)
    wqk_ext[:, D] = (-Wg[:D, 0]).astype(bf)
    wqk_ext[:, D + 1] = 1.0
    wvo_ext = np.empty((D, D + 2), dtype=bf)
    wvo_ext[:, :D] = wvo64.astype(bf)
    wvo_ext[:, D] = (-(wvo64 @ Wg[D:, 0])).astype(bf)
    wvo_ext[:, D + 1] = wvo64.sum(axis=1).astype(bf)
    identb = np.eye(P, dtype=bf)

    q_bf = query.astype(bf)
    mqt = np.empty((B, (K + 2) * D), dtype=bf)
    mqt[:, :K * D] = mem.reshape(B, K * D)
    mqt[:, K * D:(K + 1) * D] = q_bf
    # Tile-transposed q: qt[t*128 + p, c*128 + r] = q[t*128 + r, c*128 + p]
    mqt[:, (K + 1) * D:] = (
        q_bf.reshape(B // P, P, NCH, P).transpose(0, 3, 2, 1).reshape(B, D)
    )
    # Host-precomputed, tile-transposed mask penalty and 1/conf:
    # pen_T[c][p, t*K+k] = 0 if mask else -PEN ; rconf_T[c][p, t] = 1/conf
    pen_full = ((mask.astype(np.float32) - 1.0) * PEN).astype(bf)      # [B, K]
    rconf_full = (
        1.0 + np.exp(SIM_THRESH - sims.max(axis=1))
    ).astype(np.float32)                                               # [B]

    if "nc" not in _CACHE:
        _CACHE["nc"] = _build()
    nc = _CACHE["nc"]

    in_maps = []
    for c in range(N_CORES):
        sl = slice(c * ROWS, (c + 1) * ROWS)
        pen_t = np.ascontiguousarray(
            pen_full[sl].reshape(NT_FULL, P, K).transpose(1, 0, 2).reshape(P, -1)
        )
        rconf_t = np.ascontiguousarray(
            rconf_full[sl].reshape(NT_FULL, P).T
        )
        in_maps.append({
            "mqt": mqt[sl], "pen": pen_t, "rconf": rconf_t,
            "wqk": wqk_ext, "wvo": wvo_ext, "ident": identb,
        })

    from concourse.bass_utils import run_bass_kernel_spmd

    res = run_bass_kernel_spmd(nc, in_maps, list(range(N_CORES)), trace=TRACE)
    LAST_RESULTS = res
    return np.concatenate(
        [res.results[c]["o"].astype(np.float32) for c in range(N_CORES)], axis=0
    )


# revision 17
# speedup vs baseline: 1.1807x; 1.1807x over previous
"""Memory-augmented attention kernel for Trainium2 (Bass/Tile), 8-core data parallel.

Reference computation (per row b of B=32768, D=512, K=5):
    q' = query@Wq + bq
    k  = mem@Wk + bk ; v = mem@Wv + bv
    scores = (q'.k_j)/sqrt(D) masked-softmax -> w
    mem_out = (sum_j w_j v_j)@Wo + bo
    gate = sigmoid([query, mem_out]@Wg + bg); conf = sigmoid(max_sim - 0.7)
    out = LN(query + gate*conf*mem_out) * ln_g + ln_b

Algebraic refactoring (all biases zero / identity LN affine in this problem;
a numpy fallback covers the general case):
    scores_bk = m_bk . (query_b @ (Wq Wk^T / sqrt(D)))
    mem_b     = (sum_k w_bk m_bk) @ (Wv Wo)
    gate_b    = 1/(1+exp(-(q.g1 + rsum * mcomb.(WvWo g2))))

v2 design (vs the f32 baseline):
  - All HBM I/O in bf16 (q, m, host-pretransposed qT, output) -> ~2x less DMA.
  - Softmax without max-subtraction: scores are O(1); mask penalty -60.
  - mcomb computed TRANSPOSED directly on PE: matmul(lhsT=m_k chunk,
    rhs=diag(w_k)) accumulated over k gives mcT = (sum_k w_k m_k)^T without
    any separate transpose pass.
  - qT supplied by the host in tile-transposed layout -> no PE transposes at all.
  - -g1 / -(Wvo g2) folded as a 513th column of the weight mats -> gate dots
    ride along the big matmuls with the same stationary weights.
  - Per-engine per-tile budget (est): PE ~3.5us, DVE ~3.0us, ACT ~2.7us, GP low.

This container's walrus build only encodes one sync-wait per instruction;
see _install_tile_patches.
"""

import numpy as np

B, D, K = 32768, 512, 5
N_CORES = 8
ROWS = B // N_CORES        # rows per core
P = 128                    # partitions
NT_FULL = ROWS // P        # tiles per core (32)
NCH = D // P               # 128-contraction chunks (4)
SCALE = float(D) ** -0.5
PEN = 60.0                 # mask penalty (scores are O(1), exp(-55) == 0)
LN_EPS = 1e-5
SIM_THRESH = 0.7

_CACHE = {}

TRACE = False              # set by test harness to collect a HW profile
LAST_RESULTS = None        # BassKernelResults of the last run (for profiling)


def _install_tile_patches():
    """Work around two walrus limitations in this container:
    - instructions accept very few sync-wait slots: split the kernel-tail
      drain (which Tile loads with one wait per outstanding semaphore) into
      a chain of single-wait drains;
    - EVENT_SEMAPHORE_RANGE_CLEAR is not encodable: skip the on-device sem
      clear (each kernel() call executes a freshly loaded NEFF) while keeping
      the allocator bookkeeping.
    """
    import concourse.tile as tile
    from concourse.vector_clock import ScopedClock

    if getattr(tile.TileContext._drain_and_barrier, "_patched", False):
        return

    def patched(self, tick_clock, wait_clock):
        import bass_rust

        nc = self.nc
        drain_inst = nc.sync.drain()
        wait_clock.add_sem_waits(
            drain_inst.ins, ScopedClock({None: tick_clock.global_clock})
        )
        si = drain_inst.ins.sync_info
        waits = list(si.on_wait) if si is not None and si.on_wait else []
        if len(waits) > 1:
            drain_inst.ins.sync_info = bass_rust.SyncInfo(
                on_wait=waits[:1], on_update=list(si.on_update or [])
            )
            for w in waits[1:]:
                d2 = nc.sync.drain()
                d2.ins.sync_info = bass_rust.SyncInfo(on_wait=[w], on_update=[])
        nc.all_engine_barrier()
        assert self.sems is not None
        popped = nc._tile_sem_poison_stack.pop()
        assert popped is self._sem_poison
        sems = list(self.sems.allocated().values())
        sem_nums = [s.num for s in sems]
        nc._state.prepend_free_semaphores(sem_nums)
        for poison_set in nc._tile_sem_poison_stack:
            poison_set.update(sem_nums)
        nc.all_engine_barrier()

    patched._patched = True
    tile.TileContext._drain_and_barrier = patched

    # This walrus build accepts at most one sync-wait per instruction:
    # at commit time, peel off extra waits onto single-wait nop/drain
    # instructions inserted just before the owner.
    _orig_commit = tile.TileContext._commit_instruction

    def commit_patched(self, inst, lazy_reg_writes=True):
        import bass_rust
        from concourse import mybir

        si = inst.sync_info
        if si is not None and si.on_wait and len(si.on_wait) > 1:
            waits = list(si.on_wait)
            inst.sync_info = bass_rust.SyncInfo(
                on_wait=waits[-1:], on_update=list(si.on_update or [])
            )
            for w in waits[:-1]:
                eng = self.nc.engines[inst.engine]
                if not hasattr(eng, "engine_nop"):
                    nop = mybir.InstDrain(
                        name=self.nc.get_next_instruction_name(), ins=[], outs=[]
                    )
                    nop.engine = inst.engine
                else:
                    # sequencer-only ENGINE_NOP: carries the wait without
                    # flushing the compute pipeline the way a drain does
                    nop = eng.engine_nop().ins
                nop.sync_info = bass_rust.SyncInfo(on_wait=[w], on_update=[])
                self._add_instruction(nop)
        return _orig_commit(self, inst, lazy_reg_writes)

    tile.TileContext._commit_instruction = commit_patched


def _build(ntiles=NT_FULL):
    import concourse.bass as bass
    import concourse.tile as tile
    from concourse import mybir

    _install_tile_patches()

    f32 = mybir.dt.float32
    bf16 = mybir.dt.bfloat16
    u8 = mybir.dt.uint8
    AF = mybir.ActivationFunctionType
    OP = mybir.AluOpType

    rows = ntiles * P
    rD = 1.0 / float(D)

    nc = bass.Bass()
    mqt_d = nc.declare_dram_parameter(
        "mqt", [rows, (K + 2) * D], bf16, isOutput=False
    )
    pen_d = nc.declare_dram_parameter("pen", [P, ntiles * K], bf16, isOutput=False)
    rconf_d = nc.declare_dram_parameter("rconf", [P, ntiles], f32, isOutput=False)
    wqk_d = nc.declare_dram_parameter("wqk", [D, D + 2], bf16, isOutput=False)
    wvo_d = nc.declare_dram_parameter("wvo", [D, D + 2], bf16, isOutput=False)
    id_d = nc.declare_dram_parameter("ident", [P, P], bf16, isOutput=False)
    o_d = nc.declare_dram_parameter("o", [rows, D], bf16, isOutput=True)

    mqt_t = mqt_d.rearrange("(t p) d -> t p d", p=P)
    o_t = o_d.rearrange("(t p) d -> t p d", p=P)

    with tile.TileContext(nc) as tc:
        with (
            tc.tile_pool(name="consts", bufs=1) as consts,
            tc.tile_pool(name="mload", bufs=8) as mload,
            tc.tile_pool(name="work", bufs=6) as work,
            tc.tile_pool(name="smalls", bufs=9) as smalls,
            tc.tile_pool(name="ptmp", bufs=3, space="PSUM") as ptmp,
            tc.tile_pool(name="pmem", bufs=3, space="PSUM") as pmem,
            tc.tile_pool(name="pqg", bufs=1, space="PSUM") as pqg,
            tc.tile_pool(name="pmg", bufs=1, space="PSUM") as pmg,
        ):
            # ---- constants, loaded once ----
            wqk_sb = consts.tile([P, NCH, D + 2], bf16)
            nc.sync.dma_start(out=wqk_sb, in_=wqk_d.rearrange("(c p) e -> p c e", p=P))
            wvo_sb = consts.tile([P, NCH, D + 2], bf16)
            nc.sync.dma_start(out=wvo_sb, in_=wvo_d.rearrange("(c p) e -> p c e", p=P))
            identb = consts.tile([P, P], bf16)
            nc.sync.dma_start(out=identb, in_=id_d[:, :])

            pen_all = consts.tile([P, ntiles, K], bf16)
            nc.sync.dma_start(
                out=pen_all, in_=pen_d.rearrange("p (t k) -> p t k", k=K)
            )
            rconf_all = consts.tile([P, ntiles], f32)
            nc.sync.dma_start(out=rconf_all, in_=rconf_d[:, :])

            epsc = consts.tile([P, 1], f32)
            nc.vector.memset(epsc, LN_EPS)


            # Per-tile live state; 5-stage software pipeline (lag 4).
            st = {}

            def dma_in(t):
                s = st.setdefault(t, {})
                mqt = mload.tile([P, (K + 2) * D], bf16, tag="mqt", name="mqt")
                nc.sync.dma_start(out=mqt, in_=mqt_t[t])
                s["m"] = mqt[:, 0:K * D]
                s["q"] = mqt[:, K * D:(K + 1) * D]
                s["qT"] = mqt[:, (K + 1) * D:(K + 2) * D]

            def stage_a(t):
                # t' = q@Wqk (row-major, via host-transposed qT) ; nqdot = -q.g1
                s = st[t]
                psum_t = ptmp.tile([P, D], f32, tag="ptmp", name="psum_t")
                psum_qg = pqg.tile([P, 2], f32, tag="pqg", name="psum_qg")
                for c in range(NCH):
                    sl = slice(c * P, (c + 1) * P)
                    nc.tensor.matmul(
                        psum_t,
                        lhsT=s["qT"][:, sl],
                        rhs=wqk_sb[:, c, 0:D],
                        start=(c == 0), stop=(c == NCH - 1),
                    )
                    nc.tensor.matmul(
                        psum_qg,
                        lhsT=s["qT"][:, sl],
                        rhs=wqk_sb[:, c, D:D + 2],
                        start=(c == 0), stop=(c == NCH - 1),
                    )
                tb = work.tile([P, D], bf16, tag="t_bf", name="t_bf")
                nc.scalar.copy(out=tb, in_=psum_t)
                s["t_bf"] = tb
                nq = smalls.tile([P, 2], f32, tag="nqdot", name="nqdot")
                nc.scalar.copy(out=nq, in_=psum_qg)
                s["nqdot"] = nq

            def stage_b(t):
                # raw_k = m_k . t'  (5x STT with accumulate) ; scores = raw + pen
                s = st[t]
                raw = smalls.tile([P, K], f32, tag="raw", name="raw")
                scratch = work.tile([P, D], bf16, tag="scratch", name="scratch")
                for k in range(K):
                    nc.vector.scalar_tensor_tensor(
                        out=scratch,
                        in0=s["m"][:, k * D:(k + 1) * D],
                        scalar=1.0,
                        in1=s["t_bf"],
                        op0=OP.mult, op1=OP.mult,
                        accum_out=raw[:, k:k + 1],
                    )
                sc = smalls.tile([P, K], f32, tag="scores", name="scores")
                nc.vector.tensor_tensor(
                    out=sc, in0=raw, in1=pen_all[:, t, :], op=OP.add
                )
                s["scores"] = sc

            def stage_c(t):
                # w = exp(scores) (unnormalized); rsum = 1/sum(w); diag(w_k) tiles
                s = st[t]
                w = smalls.tile([P, K], f32, tag="w", name="wtile")
                sumexp = smalls.tile([P, 1], f32, tag="sumexp", name="sumexp")
                nc.scalar.activation(
                    out=w, in_=s["scores"], func=AF.Exp, accum_out=sumexp
                )
                rsum = smalls.tile([P, 1], f32, tag="rsum", name="rsum")
                nc.vector.reciprocal(out=rsum, in_=sumexp)
                s["rsum"] = rsum
                xg = smalls.tile([P, 1], f32, tag="xg", name="xg")
                nc.gpsimd.tensor_tensor(
                    out=xg, in0=sumexp, in1=rconf_all[:, t:t + 1], op=OP.mult
                )
                s["xg"] = xg
                dk = work.tile([P, K, P], bf16, tag="diag", name="diag")
                nc.gpsimd.tensor_tensor(
                    out=dk[:, :, :],
                    in0=identb[:, :].rearrange(
                        "p (o j) -> p o j", o=1).broadcast_to([P, K, P]),
                    in1=w[:, :].rearrange(
                        "p (k o) -> p k o", o=1).broadcast_to([P, K, P]),
                    op=OP.mult,
                )
                s["dk"] = dk

            def stage_d1(t):
                # mcT = (sum_k w_k m_k)^T via matmul(lhsT=m chunk, rhs=diag(w_k))
                s = st[t]
                psum_mct = ptmp.tile([P, D], f32, tag="ptmp", name="psum_mct")
                for c in range(NCH):
                    sl = slice(c * P, (c + 1) * P)
                    for k in range(K):
                        nc.tensor.matmul(
                            psum_mct[:, sl],
                            lhsT=s["m"][:, k * D + c * P: k * D + (c + 1) * P],
                            rhs=s["dk"][:, k, :],
                            start=(k == 0), stop=(k == K - 1),
                        )
                mct = work.tile([P, D], bf16, tag="mct", name="mct")
                nc.scalar.copy(out=mct, in_=psum_mct)
                s["mct"] = mct

            def stage_d2(t):
                # mem = mcomb@Wvo (+ gate dot columns); gate scalars
                s = st[t]
                mct = s["mct"]
                pm = pmem.tile([P, D], f32, tag="pmem", name="psum_mem")
                pmgt = pmg.tile([P, 2], f32, tag="pmg", name="psum_mg")
                for c in range(NCH):
                    sl = slice(c * P, (c + 1) * P)
                    nc.tensor.matmul(
                        pm,
                        lhsT=mct[:, sl],
                        rhs=wvo_sb[:, c, 0:D],
                        start=(c == 0), stop=False,
                    )
                    nc.tensor.matmul(
                        pmgt,
                        lhsT=mct[:, sl],
                        rhs=wvo_sb[:, c, D:D + 2],
                        start=(c == 0), stop=(c == NCH - 1),
                    )
                # gate: ge = exp(-(qdot + rsum*mdot));
                # 1/s = (1+ge)*sumexp/conf = ge*X + X with X = sumexp/conf
                ge = smalls.tile([P, 1], f32, tag="ge", name="ge")
                nc.scalar.activation(
                    out=ge, in_=pmgt[:, 0:1], func=AF.Exp,
                    bias=s["nqdot"][:, 0:1], scale=s["rsum"],
                )
                rs = smalls.tile([P, 1], f32, tag="rs", name="rs")
                nc.vector.tensor_scalar(
                    out=rs, in0=ge, scalar1=s["xg"][:, 0:1],
                    scalar2=s["xg"][:, 0:1], op0=OP.mult, op1=OP.add,
                )
                ds = work.tile([P, P], bf16, tag="ds", name="ds")
                nc.vector.tensor_scalar(
                    out=ds, in0=identb, scalar1=rs[:, 0:1],
                    scalar2=None, op0=OP.mult,
                )
                s["ds"] = ds
                # rowsum(x) = qsum/s + memsum, both dot-columns of the GEMMs
                rowsum = smalls.tile([P, 1], f32, tag="rowsum", name="rowsum")
                nc.vector.tensor_scalar(
                    out=rowsum, in0=rs, scalar1=s["nqdot"][:, 1:2],
                    scalar2=pmgt[:, 1:2], op0=OP.mult, op1=OP.add,
                )
                negmu2 = smalls.tile([P, 1], f32, tag="negmu2", name="negmu2")
                nc.vector.tensor_scalar(
                    out=negmu2, in0=rowsum, scalar1=rowsum[:, 0:1],
                    scalar2=-rD * rD, op0=OP.mult, op1=OP.mult,
                )
                nrw = smalls.tile([P, 1], f32, tag="nrw", name="nrw")
                nc.vector.tensor_scalar(
                    out=nrw, in0=rowsum, scalar1=-rD, scalar2=None, op0=OP.mult
                )
                s["negmu2"] = negmu2
                s["nrw"] = nrw
                s["pmem"] = pm

            def stage_e(t):
                # x = q/s + mem (in PSUM); LN(x) == LN(q + s*mem).
                # The q/s residual closes the PSUM group opened two
                # iterations earlier, decoupling PE from the gate chain.
                s = st.pop(t)
                nc.tensor.matmul(
                    s["pmem"], lhsT=s["ds"], rhs=s["q"], start=False, stop=True,
                )
                sqscr = work.tile([P, D], bf16, tag="sqscr", name="sqscr")
                sumsq = smalls.tile([P, 1], f32, tag="sumsq", name="sumsq")
                nc.scalar.activation(
                    out=sqscr, in_=s["pmem"], func=AF.Square, accum_out=sumsq
                )
                # LN tail entirely on ACT: var -> ln -> rstd -> -mu*rstd -> apply
                varc = smalls.tile([P, 1], f32, tag="varc", name="varc")
                nc.scalar.activation(
                    out=varc, in_=sumsq, func=AF.Identity,
                    bias=s["negmu2"], scale=rD,
                )
                lnv = smalls.tile([P, 1], f32, tag="lnv", name="lnv")
                nc.scalar.activation(
                    out=lnv, in_=varc, func=AF.Ln, bias=epsc, scale=1.0
                )
                rstd = smalls.tile([P, 1], f32, tag="rstd", name="rstd")
                nc.scalar.activation(out=rstd, in_=lnv, func=AF.Exp, scale=-0.5)
                nmr = smalls.tile([P, 1], f32, tag="nmr", name="nmr")
                nc.scalar.activation(
                    out=nmr, in_=rstd, func=AF.Copy, scale=s["nrw"][:, 0:1]
                )
                outf = work.tile([P, D], bf16, tag="outf", name="outf")
                nc.scalar.activation(
                    out=outf, in_=s["pmem"], func=AF.Identity,
                    bias=nmr, scale=rstd,
                )
                nc.gpsimd.dma_start(out=o_t[t], in_=outf)

            dma_in(0)
            for i in range(ntiles + 6):
                if i + 1 < ntiles:
                    dma_in(i + 1)
                if i < ntiles:
                    stage_a(i)
                if 0 <= i - 6:
                    stage_e(i - 6)
                if 0 <= i - 3 <= ntiles - 1:
                    stage_d1(i - 3)
                if 0 <= i - 2 <= ntiles - 1:
                    stage_c(i - 2)
                if 0 <= i - 1 <= ntiles - 1:
                    stage_b(i - 1)
                if 0 <= i - 4 <= ntiles - 1:
                    stage_d2(i - 4)

    return nc


def _numpy_fallback(query, retrieved_memories, similarities, mask,
                    Wq, bq, Wk, bk, Wv, bv, Wo, bo, Wg, bg, ln_g, ln_b):
    x = query.astype(np.float64)
    m = retrieved_memories.astype(np.float64)
    q = x @ Wq + bq
    k = np.einsum("bkd,de->bke", m, Wk.astype(np.float64)) + bk
    v = np.einsum("bkd,de->bke", m, Wv.astype(np.float64)) + bv
    scores = np.einsum("bd,bkd->bk", q, k) * (D ** -0.5)
    scores = np.where(mask, scores, -np.inf)
    sm = scores - scores.max(-1, keepdims=True)
    w = np.exp(sm)
    w /= w.sum(-1, keepdims=True)
    w = np.where(mask, w, 0.0)
    mem = np.einsum("bk,bkd->bd", w, v) @ Wo + bo
    gate = 1 / (1 + np.exp(-(np.concatenate([x, mem], -1) @ Wg + bg)))
    conf = 1 / (1 + np.exp(-(similarities.max(-1, keepdims=True) - SIM_THRESH)))
    out = x + (gate * conf) * mem
    mu = out.mean(-1, keepdims=True)
    var = ((out - mu) ** 2).mean(-1, keepdims=True)
    out = (out - mu) / np.sqrt(var + LN_EPS) * ln_g + ln_b
    return out.astype(np.float32)


def kernel(**inputs):
    global LAST_RESULTS
    query = np.ascontiguousarray(np.asarray(inputs["query"], dtype=np.float32))
    mem = np.ascontiguousarray(
        np.asarray(inputs["retrieved_memories"], dtype=np.float32)
    )
    sims = np.ascontiguousarray(np.asarray(inputs["similarities"], dtype=np.float32))
    mask = np.asarray(inputs["mask"])
    Wq = np.asarray(inputs["Wq"], dtype=np.float64)
    Wk = np.asarray(inputs["Wk"], dtype=np.float64)
    Wv = np.asarray(inputs["Wv"], dtype=np.float64)
    Wo = np.asarray(inputs["Wo"], dtype=np.float64)
    Wg = np.asarray(inputs["Wg"], dtype=np.float64)

    # The device kernel folds all-zero biases / identity LN affine away.
    nontrivial = (
        any(np.any(np.asarray(inputs[n])) for n in ("bq", "bk", "bv", "bo", "bg"))
        or np.any(np.asarray(inputs["ln_b"]))
        or np.any(np.asarray(inputs["ln_g"]) != 1.0)
    )
    if nontrivial or query.shape != (B, D):
        return _numpy_fallback(
            query, mem, sims, mask, Wq=Wq, bq=np.asarray(inputs["bq"]),
            Wk=Wk, bk=np.asarray(inputs["bk"]), Wv=Wv, bv=np.asarray(inputs["bv"]),
            Wo=Wo, bo=np.asarray(inputs["bo"]), Wg=Wg, bg=np.asarray(inputs["bg"]),
            ln_g=np.asarray(inputs["ln_g"]), ln_b=np.asarray(inputs["ln_b"]),
        )

    import ml_dtypes
    bf = ml_dtypes.bfloat16
    wqk64 = (Wq @ Wk.T) * (float(D) ** -0.5)
    wvo64 = Wv @ Wo
    wqk_ext = np.empty((D, D + 2), dtype=bf)
    wqk_ext[:, :D] = wqk64.astype(bf)
    wqk_ext[:, D] = (-Wg[:D, 0]).astype(bf)
    wqk_ext[:, D + 1] = 1.0
    wvo_ext = np.empty((D, D + 2), dtype=bf)
    wvo_ext[:, :D] = wvo64.astype(bf)
    wvo_ext[:, D] = (-(wvo64 @ Wg[D:, 0])).astype(bf)
    wvo_ext[:, D + 1] = wvo64.sum(axis=1).astype(bf)
    identb = np.eye(P, dtype=bf)

    q_bf = query.astype(bf)
    mqt = np.empty((B, (K + 2) * D), dtype=bf)
    mqt[:, :K * D] = mem.reshape(B, K * D)
    mqt[:, K * D:(K + 1) * D] = q_bf
    # Tile-transposed q: qt[t*128 + p, c*128 + r] = q[t*128 + r, c*128 + p]
    mqt[:, (K + 1) * D:] = (
        q_bf.reshape(B // P, P, NCH, P).transpose(0, 3, 2, 1).reshape(B, D)
    )
    # Host-precomputed, tile-transposed mask penalty and 1/conf:
    # pen_T[c][p, t*K+k] = 0 if mask else -PEN ; rconf_T[c][p, t] = 1/conf
    pen_full = ((mask.astype(np.float32) - 1.0) * PEN).astype(bf)      # [B, K]
    rconf_full = (
        1.0 + np.exp(SIM_THRESH - sims.max(axis=1))
    ).astype(np.float32)                                               # [B]

    if "nc" not in _CACHE:
        _CACHE["nc"] = _build()
    nc = _CACHE["nc"]

    in_maps = []
    for c in range(N_CORES):
        sl = slice(c * ROWS, (c + 1) * ROWS)
        pen_t = np.ascontiguousarray(
            pen_full[sl].reshape(NT_FULL, P, K).transpose(1, 0, 2).reshape(P, -1)
        )
        rconf_t = np.ascontiguousarray(
            rconf_full[sl].reshape(NT_FULL, P).T
        )
        in_maps.append({
            "mqt": mqt[sl], "pen": pen_t, "rconf": rconf_t,
            "wqk": wqk_ext, "wvo": wvo_ext, "ident": identb,
        })

    from concourse.bass_utils import run_bass_kernel_spmd

    res = run_bass_kernel_spmd(nc, in_maps, list(range(N_CORES)), trace=TRACE)
    LAST_RESULTS = res
    return np.concatenate(
        [res.results[c]["o"].astype(np.float32) for c in range(N_CORES)], axis=0
    )
